# revision 14
# baseline (speedup 1.0000x reference)
"""AnomalyDAE 4-layer GCN on 8 TRN2 NeuronCores.

Strategy (node sharding per the sharding hint):
  - Nodes partitioned contiguously across 8 cores (6250/core).
  - Per layer: local matmul h = A @ W (PE, bf16 in / f32 acc), pre-scaled by
    dinv so symmetric normalization becomes out = dinv * segsum(hs[src]),
    hs = dinv * h.  AllGather the bf16 hs table -> [N, dout] per core.
  - Message passing: edges (incl. self-loops) are sorted by destination
    tile (128 dst rows) on host.  Per tile: dma_gather the src rows (bf16)
    from the table, then segment-sum ON THE TENSOR ENGINE: for each
    128-token block, a one-hot selection matrix Seg[tok, row] =
    (dst_rel[tok] == row) is generated by the vector engine (is_equal
    against a row-iota), and matmul(Seg^T @ msg) accumulates the tile's
    [128, dout] result in PSUM.  No scatter-add (its HBM CCE add is not
    atomic for duplicate indices), no accumulator round-trip.
  - Epilogue straight from PSUM: y = dinv*psum + b, relu, PE-transpose to
    build the next layer's lhsT.
  - int16 gather indices => table split in two halves of 25000 rows; the
    d=64 layer is zero-padded to 128 features (bf16 elem size must be a
    multiple of 256 bytes).
"""

import math

import numpy as np

N_CORES = 8
P = 128
GROUP = 2  # dst tiles per gather call pair

_CACHE = {}


def _build_gcn(n_nodes, npc, npad, half, dims, layout, total_tok, n_cores):
    import concourse.bacc as bacc
    import concourse.tile as tile
    from concourse import mybir
    from concourse.library_config import mlp
    from contextlib import ExitStack

    f32 = mybir.dt.float32
    bf16 = mybir.dt.bfloat16
    i16 = mybir.dt.int16
    ntiles = npad // P

    nc = bacc.Bacc("TRN2", debug=False, num_devices=n_cores)

    din0 = dims[0][0]
    kc0 = math.ceil(din0 / P)
    xT = nc.declare_dram_parameter("xT", [din0, npad], bf16, isOutput=False)
    Ws = [
        nc.declare_dram_parameter(f"w{i}", [din, dout], bf16, isOutput=False)
        for i, (din, dout) in enumerate(dims)
    ]
    Bs = [
        nc.declare_dram_parameter(f"b{i}", [P, dout], f32, isOutput=False)
        for i, (din, dout) in enumerate(dims)
    ]
    dinv_in = nc.declare_dram_parameter("dinv", [P, ntiles], f32, isOutput=False)
    ident_in = nc.declare_dram_parameter("ident", [P, P], bf16, isOutput=False)
    riota_in = nc.declare_dram_parameter("riota", [P, P], f32, isOutput=False)
    gi_in = nc.declare_dram_parameter(
        "gidx", [P, total_tok // 16], i16, isOutput=False
    )
    drel_in = nc.declare_dram_parameter(
        "drel", [P, total_tok // P], f32, isOutput=False
    )
    dout_last = dims[-1][1]
    out_ext = nc.declare_dram_parameter("out", [npc, dout_last], f32, isOutput=True)

    hs_loc = [
        nc.dram_tensor(f"hs_loc{i}", [npc, d], bf16) for i, (_, d) in enumerate(dims)
    ]
    hs_full = [
        nc.dram_tensor(f"hs_full{i}", [n_nodes, d], bf16, addr_space="Shared")
        for i, (_, d) in enumerate(dims)
    ]

    relu = mybir.ActivationFunctionType.Relu
    copyf = mybir.ActivationFunctionType.Copy
    mult = mybir.AluOpType.mult
    add = mybir.AluOpType.add
    iseq = mybir.AluOpType.is_equal

    with tile.TileContext(nc) as tc, ExitStack() as ctx:
        const = ctx.enter_context(tc.tile_pool(name="const", bufs=1))
        at_pool = ctx.enter_context(tc.tile_pool(name="acts", bufs=2))
        work = ctx.enter_context(tc.tile_pool(name="work", bufs=4))
        msgp = ctx.enter_context(tc.tile_pool(name="msg", bufs=4))
        idxp = ctx.enter_context(tc.tile_pool(name="idx", bufs=4))
        segp = ctx.enter_context(tc.tile_pool(name="seg", bufs=6))
        psum = ctx.enter_context(tc.tile_pool(name="psum", bufs=2, space="PSUM"))
        psacc = ctx.enter_context(tc.tile_pool(name="psacc", bufs=2, space="PSUM"))

        nc.gpsimd.load_library(mlp)

        ident = const.tile([P, P], bf16)
        nc.sync.dma_start(out=ident[:], in_=ident_in[:, :])
        riota = const.tile([P, P], f32)
        nc.sync.dma_start(out=riota[:], in_=riota_in[:, :])
        dinv_sb = const.tile([P, ntiles], f32)
        nc.sync.dma_start(out=dinv_sb[:], in_=dinv_in[:])
        drel_sb = const.tile([P, total_tok // P], f32)
        nc.sync.dma_start(out=drel_sb[:], in_=drel_in[:, :])

        w_sb, b_sb = [], []
        for i, (din, dout) in enumerate(dims):
            kcs = math.ceil(din / P)
            wi = []
            for kc in range(kcs):
                rows = min(P, din - kc * P)
                wt = const.tile([rows, dout], bf16, tag=f"w{i}_{kc}")
                nc.sync.dma_start(out=wt[:], in_=Ws[i][kc * P : kc * P + rows, :])
                wi.append(wt)
            w_sb.append(wi)
            bt = const.tile([P, dout], f32, tag=f"b{i}")
            nc.sync.dma_start(out=bt[:], in_=Bs[i][:, :])
            b_sb.append(bt)

        aT = []
        for kc in range(kc0):
            rows = min(P, din0 - kc * P)
            t_ = at_pool.tile([rows, npad], bf16, tag=f"aT_{kc}", name=f"aT0_{kc}")
            nc.sync.dma_start(out=t_[:], in_=xT[kc * P : kc * P + rows, :])
            aT.append(t_)

        n_layers = len(dims)
        for li, (din, dout) in enumerate(dims):
            last = li == n_layers - 1
            kcs = len(aT)

            # ---- local matmul + dinv pre-scale -> bf16 hs table shard ----
            for t in range(ntiles):
                ps = psum.tile([P, dout], f32, tag="mm", name="mm")
                for kc in range(kcs):
                    nc.tensor.matmul(
                        ps[:],
                        aT[kc][:, t * P : (t + 1) * P],
                        w_sb[li][kc][:],
                        start=(kc == 0),
                        stop=(kc == kcs - 1),
                    )
                hs_t = work.tile([P, dout], bf16, tag="hs", name="hs")
                nc.scalar.activation(
                    hs_t[:], ps[:], copyf, bias=0.0, scale=dinv_sb[:, t : t + 1]
                )
                r0 = t * P
                r1 = min((t + 1) * P, npc)
                nc.sync.dma_start(out=hs_loc[li][r0:r1, :], in_=hs_t[0 : r1 - r0, :])

            # ---- AllGather the bf16 table ----
            nc.gpsimd.collective_compute(
                "AllGather",
                mybir.AluOpType.bypass,
                replica_groups=[list(range(n_cores))],
                ins=[hs_loc[li][:, :]],
                outs=[hs_full[li][:, :]],
            )

            # ---- gather + PE segment-sum per dst tile group ----
            if not last:
                kcs_next = math.ceil(dout / P)
                aT_next = []
                for kc in range(kcs_next):
                    rows = min(P, dout - kc * P)
                    aT_next.append(
                        at_pool.tile(
                            [rows, npad],
                            bf16,
                            tag=f"aT_{kc}",
                            name=f"aT{li + 1}_{kc}",
                        )
                    )

            for grp in layout:
                msgs = {}
                for h, c0, c1 in grp["calls"]:
                    ntok = c1 - c0
                    idxt = idxp.tile([P, ntok // 16], i16, tag="idxt", name="idxt")
                    nc.sync.dma_start(
                        out=idxt[:], in_=gi_in[:, c0 // 16 : c1 // 16]
                    )
                    msg = msgp.tile([P, ntok // P, dout], bf16, tag="msg", name="msg")
                    lo = h * half
                    rows_h = half if h == 0 else n_nodes - half
                    nc.gpsimd.dma_gather(
                        msg[:],
                        hs_full[li][lo : lo + rows_h, :],
                        idxt[:],
                        ntok,
                        ntok,
                        dout,
                        single_packet=False,
                    )
                    msgs[h] = (msg, c0 // P)

                for t, branges in grp["tiles"]:
                    blocks = [
                        (h, j)
                        for h, j0, j1 in branges
                        for j in range(j0, j1)
                    ]
                    pacc = psacc.tile([P, dout], f32, tag="segacc", name="segacc")
                    for bi, (h, j) in enumerate(blocks):
                        seg = segp.tile([P, P], bf16, tag="seg", name="seg")
                        nc.vector.tensor_scalar(
                            seg[:], riota[:], drel_sb[:, j : j + 1], None, iseq
                        )
                        msg, base = msgs[h]
                        nc.tensor.matmul(
                            pacc[:],
                            seg[:],
                            msg[:, j - base, :],
                            start=(bi == 0),
                            stop=(bi == len(blocks) - 1),
                        )

                    # ---- epilogue straight from PSUM ----
                    y = work.tile([P, dout], f32, tag="y", name="y")
                    nc.vector.scalar_tensor_tensor(
                        y[:], pacc[:], dinv_sb[:, t : t + 1], b_sb[li][:], mult, add
                    )
                    if last:
                        r0 = t * P
                        r1 = min((t + 1) * P, npc)
                        nc.sync.dma_start(
                            out=out_ext[r0:r1, :], in_=y[0 : r1 - r0, :]
                        )
                    else:
                        a_t = work.tile([P, dout], bf16, tag="a", name="a")
                        nc.scalar.activation(a_t[:], y[:], relu)
                        for kc in range(kcs_next):
                            wcols = min(P, dout - kc * P)
                            pt = psum.tile([wcols, P], bf16, tag="tr", name="tr")
                            nc.tensor.transpose(
                                pt[:], a_t[:, kc * P : kc * P + wcols], ident[:]
                            )
                            nc.scalar.copy(
                                aT_next[kc][:, t * P : (t + 1) * P], pt[:]
                            )
            if not last:
                aT = aT_next

    nc.compile()
    return nc


def _preprocess(x, edge_index, n_nodes, npc, npad, half, n_cores, dims):
    import ml_dtypes

    src = np.asarray(edge_index[0], dtype=np.int64)
    dst = np.asarray(edge_index[1], dtype=np.int64)
    deg = np.bincount(dst, minlength=n_nodes).astype(np.float32) + 1.0
    dinv = (1.0 / np.sqrt(deg)).astype(np.float32)

    ntiles = npad // P
    per_core = []
    for i in range(n_cores):
        lo = i * npc
        sel = (dst >= lo) & (dst < lo + npc)
        s = np.concatenate([src[sel], np.arange(lo, lo + npc, dtype=np.int64)])
        dr = np.concatenate([dst[sel] - lo, np.arange(npc, dtype=np.int64)])
        tl = dr // P
        hh = (s >= half).astype(np.int64)
        order = np.lexsort((s, hh, tl))
        s, dr, hh, tl = s[order], dr[order], hh[order], tl[order]
        key = tl * 2 + hh
        starts = np.searchsorted(key, np.arange(ntiles * 2), "left")
        ends = np.searchsorted(key, np.arange(ntiles * 2), "right")
        per_core.append((s, dr, starts, ends))

    seglen = np.zeros((ntiles, 2), np.int64)
    for s, dr, starts, ends in per_core:
        ln = (ends - starts).reshape(ntiles, 2)
        seglen = np.maximum(seglen, ln)
    seglen = ((seglen + P - 1) // P) * P

    # stream layout: groups of GROUP tiles, within a group h0 segments then h1
    layout = []
    pos = 0
    for g0 in range(0, ntiles, GROUP):
        tl_list = list(range(g0, min(g0 + GROUP, ntiles)))
        calls = []
        tiles = [[t, []] for t in tl_list]
        for h in (0, 1):
            c0 = pos
            for k, t in enumerate(tl_list):
                L = int(seglen[t, h])
                if L:
                    tiles[k][1].append((h, pos // P, (pos + L) // P))
                pos += L
            if pos > c0:
                calls.append((h, c0, pos))
        layout.append({"calls": calls, "tiles": [(t, br) for t, br in tiles]})
    total_tok = pos

    in_maps = []
    for i in range(n_cores):
        s, dr, starts, ends = per_core[i]
        gidx = np.zeros(total_tok, np.int16)
        drel = np.full(total_tok, -1.0, np.float32)
        for grp in layout:
            for t, branges in grp["tiles"]:
                for h, j0, j1 in branges:
                    st, en = starts[t * 2 + h], ends[t * 2 + h]
                    n = en - st
                    p0 = j0 * P
                    gidx[p0 : p0 + n] = (s[st:en] - h * half).astype(np.int16)
                    drel[p0 : p0 + n] = (dr[st:en] - t * P).astype(np.float32)
        lo = i * npc
        x_loc = np.asarray(x[lo : lo + npc], dtype=np.float32)
        xT = np.zeros((x.shape[1], npad), dtype=ml_dtypes.bfloat16)
        xT[:, :npc] = x_loc.T.astype(ml_dtypes.bfloat16)
        dv = np.ones(npad, dtype=np.float32)
        dv[:npc] = dinv[lo : lo + npc]
        in_maps.append(
            {
                "xT": xT,
                "ident": np.eye(P, dtype=ml_dtypes.bfloat16),
                "riota": np.broadcast_to(
                    np.arange(P, dtype=np.float32), (P, P)
                ).copy(),
                "dinv": np.ascontiguousarray(dv.reshape(ntiles, P).T),
                "gidx": np.tile(
                    np.ascontiguousarray(gidx.reshape(total_tok // 16, 16).T),
                    (8, 1),
                ),
                "drel": np.ascontiguousarray(
                    drel.reshape(total_tok // P, P).T
                ),
            }
        )
    return in_maps, layout, total_tok, dinv


def _pad_w(w, din_p, dout_p):
    out = np.zeros((din_p, dout_p), np.float32)
    out[: w.shape[0], : w.shape[1]] = w
    return out


def kernel(x, edge_index, W1, b1, W2, b2, W3, b3, W4, b4, **_unused):
    import ml_dtypes
    from concourse.bass_utils import run_bass_kernel_spmd

    x = np.asarray(x, dtype=np.float32)
    n_nodes = x.shape[0]
    npc = n_nodes // N_CORES
    ntiles = math.ceil(npc / P)
    npad = ntiles * P
    half = (n_nodes + 1) // 2

    ws_raw = [np.asarray(w, np.float32) for w in (W1, W2, W3, W4)]
    bs_raw = [np.asarray(b, np.float32) for b in (b1, b2, b3, b4)]
    # pad every dim (except the first input / last output) to a multiple
    # of 128 so bf16 gather elem sizes stay multiples of 256B
    d_in = [ws_raw[0].shape[0]] + [
        max(P, math.ceil(w.shape[1] / P) * P) for w in ws_raw[:-1]
    ]
    d_last = max(P, math.ceil(ws_raw[-1].shape[1] / P) * P)
    d_out = d_in[1:] + [d_last]
    dims = list(zip(d_in, d_out))
    dout_raw = ws_raw[-1].shape[1]
    ws = [
        _pad_w(w, di, do).astype(ml_dtypes.bfloat16)
        for w, (di, do) in zip(ws_raw, dims)
    ]
    bs = [
        np.pad(b, (0, do - b.shape[0])).astype(np.float32)
        for b, (_, do) in zip(bs_raw, dims)
    ]

    in_maps, layout, total_tok, _ = _preprocess(
        x, edge_index, n_nodes, npc, npad, half, N_CORES, dims
    )
    key = (n_nodes, tuple(dims), total_tok)
    if key not in _CACHE:
        _CACHE[key] = _build_gcn(
            n_nodes, npc, npad, half, dims, layout, total_tok, N_CORES
        )
    nc = _CACHE[key]

    for m in in_maps:
        for i in range(4):
            m[f"w{i}"] = ws[i]
            m[f"b{i}"] = np.broadcast_to(bs[i], (P, bs[i].shape[0])).copy()

    import os

    if os.environ.get("GCN_SIM"):
        from concourse.bass_interp import MultiCoreSim

        sim = MultiCoreSim(nc, N_CORES)
        for i in range(N_CORES):
            for k, v in in_maps[i].items():
                sim.cores[i].tensor(k)[:] = v
        sim.simulate(check_with_hw=False)
        return np.concatenate(
            [sim.cores[i].mem_tensor("out") for i in range(N_CORES)], axis=0
        )[:, :dout_raw]

    res = run_bass_kernel_spmd(nc, in_maps, core_ids=list(range(N_CORES)))
    return np.concatenate(
        [res.results[i]["out"] for i in range(N_CORES)], axis=0
    )[:, :dout_raw]


# revision 18
# speedup vs baseline: 1004.2177x; 1004.2177x over previous
"""AnomalyDAE 4-layer GCN on 8 TRN2 NeuronCores.

Strategy (node sharding per the sharding hint):
  - Nodes partitioned contiguously across 8 cores (6250/core).
  - Per layer: local matmul h = A @ W (PE, bf16 in / f32 acc), pre-scaled by
    dinv so symmetric normalization becomes out = dinv * segsum(hs[src]),
    hs = dinv * h.  AllGather the bf16 hs table -> [N, dout] per core.
  - Message passing: edges (incl. self-loops) are sorted by destination
    tile (128 dst rows) on host.  Per tile: dma_gather the src rows (bf16)
    from the table, then segment-sum ON THE TENSOR ENGINE: for each
    128-token block, a one-hot selection matrix Seg[tok, row] =
    (dst_rel[tok] == row) is generated by the vector engine (is_equal
    against a row-iota), and matmul(Seg^T @ msg) accumulates the tile's
    [128, dout] result in PSUM.  No scatter-add (its HBM CCE add is not
    atomic for duplicate indices), no accumulator round-trip.
  - Epilogue straight from PSUM: y = dinv*psum + b, relu, PE-transpose to
    build the next layer's lhsT.
  - int16 gather indices => table split in two halves of 25000 rows; the
    d=64 layer is zero-padded to 128 features (bf16 elem size must be a
    multiple of 256 bytes).
"""

import math

import numpy as np

N_CORES = 8
P = 128
GROUP = 2  # dst tiles per gather call pair

_CACHE = {}
LAST_EXEC_NS = None


def _build_gcn(n_nodes, npc, npad, half, dims, layout, total_tok, n_cores):
    import concourse.bacc as bacc
    import concourse.tile as tile
    from concourse import mybir
    from concourse.library_config import mlp
    from contextlib import ExitStack

    f32 = mybir.dt.float32
    bf16 = mybir.dt.bfloat16
    i16 = mybir.dt.int16
    ntiles = npad // P

    nc = bacc.Bacc(
        "TRN2", debug=False, num_devices=n_cores, num_swdge_queues=4
    )

    din0 = dims[0][0]
    kc0 = math.ceil(din0 / P)
    xT = nc.declare_dram_parameter("xT", [din0, npad], bf16, isOutput=False)
    Ws = [
        nc.declare_dram_parameter(f"w{i}", [din, dout], bf16, isOutput=False)
        for i, (din, dout) in enumerate(dims)
    ]
    Bs = [
        nc.declare_dram_parameter(f"b{i}", [P, dout], f32, isOutput=False)
        for i, (din, dout) in enumerate(dims)
    ]
    dinv_in = nc.declare_dram_parameter("dinv", [P, ntiles], f32, isOutput=False)
    ident_in = nc.declare_dram_parameter("ident", [P, P], bf16, isOutput=False)
    nbmax = max(
        (j1 - j0)
        for grp in layout
        for _, branges in grp["tiles"]
        for _, j0, j1 in branges
    )
    riota_in = nc.declare_dram_parameter(
        "riota", [P, nbmax, P], bf16, isOutput=False
    )
    gi_in = nc.declare_dram_parameter(
        "gidx", [P, total_tok // 16], i16, isOutput=False
    )
    drel_in = nc.declare_dram_parameter(
        "drel", [P, total_tok // P], bf16, isOutput=False
    )
    dout_last = dims[-1][1]
    out_ext = nc.declare_dram_parameter("out", [npc, dout_last], f32, isOutput=True)

    hs_loc = [
        nc.dram_tensor(f"hs_loc{i}", [npc, d], bf16) for i, (_, d) in enumerate(dims)
    ]
    hs_full = [
        nc.dram_tensor(f"hs_full{i}", [n_nodes, d], bf16, addr_space="Shared")
        for i, (_, d) in enumerate(dims)
    ]

    relu = mybir.ActivationFunctionType.Relu
    copyf = mybir.ActivationFunctionType.Copy
    mult = mybir.AluOpType.mult
    add = mybir.AluOpType.add
    iseq = mybir.AluOpType.is_equal

    with tile.TileContext(nc) as tc, ExitStack() as ctx:
        const = ctx.enter_context(tc.tile_pool(name="const", bufs=1))
        at_pool = ctx.enter_context(tc.tile_pool(name="acts", bufs=2))
        work = ctx.enter_context(tc.tile_pool(name="work", bufs=4))
        msgp = ctx.enter_context(tc.tile_pool(name="msg", bufs=4))
        idxp = ctx.enter_context(tc.tile_pool(name="idx", bufs=4))
        segp = ctx.enter_context(tc.tile_pool(name="seg", bufs=6))
        psum = ctx.enter_context(tc.tile_pool(name="psum", bufs=2, space="PSUM"))
        psacc = ctx.enter_context(tc.tile_pool(name="psacc", bufs=2, space="PSUM"))

        nc.gpsimd.load_library(mlp)

        ident = const.tile([P, P], bf16)
        nc.sync.dma_start(out=ident[:], in_=ident_in[:, :])
        riota = const.tile([P, nbmax, P], bf16)
        nc.sync.dma_start(out=riota[:], in_=riota_in[:, :, :])
        dinv_sb = const.tile([P, ntiles], f32)
        nc.sync.dma_start(out=dinv_sb[:], in_=dinv_in[:])
        drel_sb = const.tile([P, total_tok // P], bf16)
        nc.sync.dma_start(out=drel_sb[:], in_=drel_in[:, :])

        w_sb, b_sb = [], []
        for i, (din, dout) in enumerate(dims):
            kcs = math.ceil(din / P)
            wi = []
            for kc in range(kcs):
                rows = min(P, din - kc * P)
                wt = const.tile([rows, dout], bf16, tag=f"w{i}_{kc}")
                nc.sync.dma_start(out=wt[:], in_=Ws[i][kc * P : kc * P + rows, :])
                wi.append(wt)
            w_sb.append(wi)
            bt = const.tile([P, dout], f32, tag=f"b{i}")
            nc.sync.dma_start(out=bt[:], in_=Bs[i][:, :])
            b_sb.append(bt)

        aT = []
        for kc in range(kc0):
            rows = min(P, din0 - kc * P)
            t_ = at_pool.tile([rows, npad], bf16, tag=f"aT_{kc}", name=f"aT0_{kc}")
            nc.sync.dma_start(out=t_[:], in_=xT[kc * P : kc * P + rows, :])
            aT.append(t_)

        n_layers = len(dims)
        for li, (din, dout) in enumerate(dims):
            last = li == n_layers - 1
            kcs = len(aT)

            # ---- local matmul + dinv pre-scale -> bf16 hs table shard ----
            for t in range(ntiles):
                ps = psum.tile([P, dout], f32, tag="mm", name="mm")
                for kc in range(kcs):
                    nc.tensor.matmul(
                        ps[:],
                        aT[kc][:, t * P : (t + 1) * P],
                        w_sb[li][kc][:],
                        start=(kc == 0),
                        stop=(kc == kcs - 1),
                    )
                hs_t = work.tile([P, dout], bf16, tag="hs", name="hs")
                nc.scalar.activation(
                    hs_t[:], ps[:], copyf, bias=0.0, scale=dinv_sb[:, t : t + 1]
                )
                r0 = t * P
                r1 = min((t + 1) * P, npc)
                nc.sync.dma_start(out=hs_loc[li][r0:r1, :], in_=hs_t[0 : r1 - r0, :])

            # ---- AllGather the bf16 table ----
            nc.gpsimd.collective_compute(
                "AllGather",
                mybir.AluOpType.bypass,
                replica_groups=[list(range(n_cores))],
                ins=[hs_loc[li][:, :]],
                outs=[hs_full[li][:, :]],
            )

            # ---- gather + PE segment-sum per dst tile group ----
            if not last:
                kcs_next = math.ceil(dout / P)
                aT_next = []
                for kc in range(kcs_next):
                    rows = min(P, dout - kc * P)
                    aT_next.append(
                        at_pool.tile(
                            [rows, npad],
                            bf16,
                            tag=f"aT_{kc}",
                            name=f"aT{li + 1}_{kc}",
                        )
                    )

            qn = 0
            for grp in layout:
                msgs = {}
                for h, c0, c1 in grp["calls"]:
                    qn = (qn + 1) % 4
                    ntok = c1 - c0
                    idxt = idxp.tile([P, ntok // 16], i16, tag="idxt", name="idxt")
                    nc.sync.dma_start(
                        out=idxt[:], in_=gi_in[:, c0 // 16 : c1 // 16]
                    )
                    msg = msgp.tile([P, ntok // P, dout], bf16, tag="msg", name="msg")
                    lo = h * half
                    rows_h = half if h == 0 else n_nodes - half
                    nc.gpsimd.dma_gather(
                        msg[:],
                        hs_full[li][lo : lo + rows_h, :],
                        idxt[:],
                        ntok,
                        ntok,
                        dout,
                        single_packet=False,
                        queue_num=qn,
                    )
                    msgs[h] = (msg, c0 // P)

                for t, branges in grp["tiles"]:
                    pacc = psacc.tile([P, dout], f32, tag="segacc", name="segacc")
                    segs = []
                    for h, j0, j1 in branges:
                        nblk = j1 - j0
                        segm = segp.tile(
                            [P, nblk, P], bf16, tag="seg", name="seg"
                        )
                        nc.vector.tensor_tensor(
                            out=segm[:],
                            in0=drel_sb[:, j0:j1].to_broadcast([P, nblk, P]),
                            in1=riota[:, 0:nblk, :],
                            op=iseq,
                        )
                        segs.append((h, j0, j1, segm))
                    nb_tot = sum(j1 - j0 for _, j0, j1, _ in segs)
                    bi = 0
                    for h, j0, j1, segm in segs:
                        msg, base = msgs[h]
                        for j in range(j0, j1):
                            nc.tensor.matmul(
                                pacc[:],
                                segm[:, j - j0, :],
                                msg[:, j - base, :],
                                start=(bi == 0),
                                stop=(bi == nb_tot - 1),
                            )
                            bi += 1

                    # ---- epilogue straight from PSUM ----
                    y = work.tile([P, dout], f32, tag="y", name="y")
                    nc.vector.scalar_tensor_tensor(
                        y[:], pacc[:], dinv_sb[:, t : t + 1], b_sb[li][:], mult, add
                    )
                    if last:
                        r0 = t * P
                        r1 = min((t + 1) * P, npc)
                        nc.sync.dma_start(
                            out=out_ext[r0:r1, :], in_=y[0 : r1 - r0, :]
                        )
                    else:
                        a_t = work.tile([P, dout], bf16, tag="a", name="a")
                        nc.scalar.activation(a_t[:], y[:], relu)
                        for kc in range(kcs_next):
                            wcols = min(P, dout - kc * P)
                            pt = psum.tile([wcols, P], bf16, tag="tr", name="tr")
                            nc.tensor.transpose(
                                pt[:], a_t[:, kc * P : kc * P + wcols], ident[:]
                            )
                            nc.scalar.copy(
                                aT_next[kc][:, t * P : (t + 1) * P], pt[:]
                            )
            if not last:
                aT = aT_next

    nc.compile()
    return nc


def _preprocess(x, edge_index, n_nodes, npc, npad, half, n_cores, dims):
    import ml_dtypes

    src = np.asarray(edge_index[0], dtype=np.int64)
    dst = np.asarray(edge_index[1], dtype=np.int64)
    deg = np.bincount(dst, minlength=n_nodes).astype(np.float32) + 1.0
    dinv = (1.0 / np.sqrt(deg)).astype(np.float32)

    ntiles = npad // P
    per_core = []
    for i in range(n_cores):
        lo = i * npc
        sel = (dst >= lo) & (dst < lo + npc)
        s = np.concatenate([src[sel], np.arange(lo, lo + npc, dtype=np.int64)])
        dr = np.concatenate([dst[sel] - lo, np.arange(npc, dtype=np.int64)])
        tl = dr // P
        hh = (s >= half).astype(np.int64)
        order = np.lexsort((s, hh, tl))
        s, dr, hh, tl = s[order], dr[order], hh[order], tl[order]
        key = tl * 2 + hh
        starts = np.searchsorted(key, np.arange(ntiles * 2), "left")
        ends = np.searchsorted(key, np.arange(ntiles * 2), "right")
        per_core.append((s, dr, starts, ends))

    seglen = np.zeros((ntiles, 2), np.int64)
    for s, dr, starts, ends in per_core:
        ln = (ends - starts).reshape(ntiles, 2)
        seglen = np.maximum(seglen, ln)
    seglen = ((seglen + P - 1) // P) * P

    # stream layout: groups of GROUP tiles, within a group h0 segments then h1
    layout = []
    pos = 0
    for g0 in range(0, ntiles, GROUP):
        tl_list = list(range(g0, min(g0 + GROUP, ntiles)))
        calls = []
        tiles = [[t, []] for t in tl_list]
        for h in (0, 1):
            c0 = pos
            for k, t in enumerate(tl_list):
                L = int(seglen[t, h])
                if L:
                    tiles[k][1].append((h, pos // P, (pos + L) // P))
                pos += L
            if pos > c0:
                calls.append((h, c0, pos))
        layout.append({"calls": calls, "tiles": [(t, br) for t, br in tiles]})
    total_tok = pos

    nbmax = max(
        (j1 - j0)
        for grp in layout
        for _, branges in grp["tiles"]
        for _, j0, j1 in branges
    )
    in_maps = []
    for i in range(n_cores):
        s, dr, starts, ends = per_core[i]
        gidx = np.zeros(total_tok, np.int16)
        drel = np.full(total_tok, -1.0, np.float32)
        for grp in layout:
            for t, branges in grp["tiles"]:
                for h, j0, j1 in branges:
                    st, en = starts[t * 2 + h], ends[t * 2 + h]
                    n = en - st
                    p0 = j0 * P
                    gidx[p0 : p0 + n] = (s[st:en] - h * half).astype(np.int16)
                    drel[p0 : p0 + n] = (dr[st:en] - t * P).astype(np.float32)
        lo = i * npc
        x_loc = np.asarray(x[lo : lo + npc], dtype=np.float32)
        xT = np.zeros((x.shape[1], npad), dtype=ml_dtypes.bfloat16)
        xT[:, :npc] = x_loc.T.astype(ml_dtypes.bfloat16)
        dv = np.ones(npad, dtype=np.float32)
        dv[:npc] = dinv[lo : lo + npc]
        in_maps.append(
            {
                "xT": xT,
                "ident": np.eye(P, dtype=ml_dtypes.bfloat16),
                "riota": np.broadcast_to(
                    np.arange(P, dtype=np.float32), (P, nbmax, P)
                ).astype(ml_dtypes.bfloat16),
                "dinv": np.ascontiguousarray(dv.reshape(ntiles, P).T),
                "gidx": np.tile(
                    np.ascontiguousarray(gidx.reshape(total_tok // 16, 16).T),
                    (8, 1),
                ),
                "drel": np.ascontiguousarray(
                    drel.reshape(total_tok // P, P).T
                ).astype(ml_dtypes.bfloat16),
            }
        )
    return in_maps, layout, total_tok, dinv


def _pad_w(w, din_p, dout_p):
    out = np.zeros((din_p, dout_p), np.float32)
    out[: w.shape[0], : w.shape[1]] = w
    return out


def kernel(x, edge_index, W1, b1, W2, b2, W3, b3, W4, b4, **_unused):
    import ml_dtypes
    from concourse.bass_utils import run_bass_kernel_spmd

    x = np.asarray(x, dtype=np.float32)
    n_nodes = x.shape[0]
    npc = n_nodes // N_CORES
    ntiles = math.ceil(npc / P)
    npad = ntiles * P
    half = (n_nodes + 1) // 2

    ws_raw = [np.asarray(w, np.float32) for w in (W1, W2, W3, W4)]
    bs_raw = [np.asarray(b, np.float32) for b in (b1, b2, b3, b4)]
    # pad every dim (except the first input / last output) to a multiple
    # of 128 so bf16 gather elem sizes stay multiples of 256B
    d_in = [ws_raw[0].shape[0]] + [
        max(P, math.ceil(w.shape[1] / P) * P) for w in ws_raw[:-1]
    ]
    d_last = max(P, math.ceil(ws_raw[-1].shape[1] / P) * P)
    d_out = d_in[1:] + [d_last]
    dims = list(zip(d_in, d_out))
    dout_raw = ws_raw[-1].shape[1]
    ws = [
        _pad_w(w, di, do).astype(ml_dtypes.bfloat16)
        for w, (di, do) in zip(ws_raw, dims)
    ]
    bs = [
        np.pad(b, (0, do - b.shape[0])).astype(np.float32)
        for b, (_, do) in zip(bs_raw, dims)
    ]

    in_maps, layout, total_tok, _ = _preprocess(
        x, edge_index, n_nodes, npc, npad, half, N_CORES, dims
    )
    key = (n_nodes, tuple(dims), total_tok)
    if key not in _CACHE:
        _CACHE[key] = _build_gcn(
            n_nodes, npc, npad, half, dims, layout, total_tok, N_CORES
        )
    nc = _CACHE[key]

    for m in in_maps:
        for i in range(4):
            m[f"w{i}"] = ws[i]
            m[f"b{i}"] = np.broadcast_to(bs[i], (P, bs[i].shape[0])).copy()

    import os

    if os.environ.get("GCN_SIM"):
        from concourse.bass_interp import MultiCoreSim

        sim = MultiCoreSim(nc, N_CORES)
        for i in range(N_CORES):
            for k, v in in_maps[i].items():
                sim.cores[i].tensor(k)[:] = v
        sim.simulate(check_with_hw=False)
        return np.concatenate(
            [sim.cores[i].mem_tensor("out") for i in range(N_CORES)], axis=0
        )[:, :dout_raw]

    trace = bool(os.environ.get("GCN_TRACE"))
    res = run_bass_kernel_spmd(
        nc, in_maps, core_ids=list(range(N_CORES)), trace=trace
    )
    global LAST_EXEC_NS
    LAST_EXEC_NS = res.exec_time_ns
    return np.concatenate(
        [res.results[i]["out"] for i in range(N_CORES)], axis=0
    )[:, :dout_raw]


# revision 22
# speedup vs baseline: 1015.6039x; 1.0113x over previous
"""AnomalyDAE 4-layer GCN on 8 TRN2 NeuronCores.

Strategy (node sharding per the sharding hint):
  - Nodes partitioned contiguously across 8 cores (6250/core).
  - Per layer: local matmul h = A @ W (PE, bf16 in / f32 acc), pre-scaled by
    dinv so symmetric normalization becomes out = dinv * segsum(hs[src]),
    hs = dinv * h.  AllGather the bf16 hs table -> [N, dout] per core.
  - Message passing: edges (incl. self-loops) are sorted by destination
    tile (128 dst rows) on host.  Per tile: dma_gather the src rows (bf16)
    from the table, then segment-sum ON THE TENSOR ENGINE: for each
    128-token block, a one-hot selection matrix Seg[tok, row] =
    (dst_rel[tok] == row) is generated by the vector engine (is_equal
    against a row-iota), and matmul(Seg^T @ msg) accumulates the tile's
    [128, dout] result in PSUM.  No scatter-add (its HBM CCE add is not
    atomic for duplicate indices), no accumulator round-trip.
  - Epilogue straight from PSUM: y = dinv*psum + b, relu, PE-transpose to
    build the next layer's lhsT.
  - int16 gather indices => table split in two halves of 25000 rows; the
    d=64 layer is zero-padded to 128 features (bf16 elem size must be a
    multiple of 256 bytes).
"""

import math

import numpy as np

N_CORES = 8
P = 128
GROUP = 2  # dst tiles per gather call pair

_CACHE = {}
LAST_EXEC_NS = None


def _build_gcn(n_nodes, npc, npad, half, dims, layout, total_tok, n_cores):
    import concourse.bacc as bacc
    import concourse.tile as tile
    from concourse import mybir
    from concourse.library_config import mlp
    from contextlib import ExitStack

    f32 = mybir.dt.float32
    bf16 = mybir.dt.bfloat16
    i16 = mybir.dt.int16
    ntiles = npad // P

    nc = bacc.Bacc(
        "TRN2", debug=False, num_devices=n_cores, num_swdge_queues=4
    )

    din0 = dims[0][0]
    kc0 = math.ceil(din0 / P)
    xT = nc.declare_dram_parameter("xT", [din0, npad], bf16, isOutput=False)
    Ws = [
        nc.declare_dram_parameter(f"w{i}", [din, dout], bf16, isOutput=False)
        for i, (din, dout) in enumerate(dims)
    ]
    Bs = [
        nc.declare_dram_parameter(f"b{i}", [P, dout], f32, isOutput=False)
        for i, (din, dout) in enumerate(dims)
    ]
    dinv_in = nc.declare_dram_parameter("dinv", [P, ntiles], f32, isOutput=False)
    ident_in = nc.declare_dram_parameter("ident", [P, P], bf16, isOutput=False)
    nbmax = max(
        (j1 - j0)
        for grp in layout
        for _, branges in grp["tiles"]
        for _, j0, j1 in branges
    )
    riota_in = nc.declare_dram_parameter(
        "riota", [P, nbmax, P], bf16, isOutput=False
    )
    gi_in = nc.declare_dram_parameter(
        "gidx", [P, total_tok // 16], i16, isOutput=False
    )
    drel_in = nc.declare_dram_parameter(
        "drel", [P, total_tok // P], bf16, isOutput=False
    )
    dout_last = dims[-1][1]
    out_ext = nc.declare_dram_parameter("out", [npc, dout_last], f32, isOutput=True)

    hs_loc = [
        nc.dram_tensor(f"hs_loc{i}", [npc, d], bf16) for i, (_, d) in enumerate(dims)
    ]
    hs_full = [
        nc.dram_tensor(f"hs_full{i}", [n_nodes, d], bf16, addr_space="Shared")
        for i, (_, d) in enumerate(dims)
    ]

    relu = mybir.ActivationFunctionType.Relu
    copyf = mybir.ActivationFunctionType.Copy
    mult = mybir.AluOpType.mult
    add = mybir.AluOpType.add
    iseq = mybir.AluOpType.is_equal

    with tile.TileContext(nc) as tc, ExitStack() as ctx:
        const = ctx.enter_context(tc.tile_pool(name="const", bufs=1))
        at_pool = ctx.enter_context(tc.tile_pool(name="acts", bufs=2))
        work = ctx.enter_context(tc.tile_pool(name="work", bufs=4))
        msgp = ctx.enter_context(tc.tile_pool(name="msg", bufs=4))
        idxp = ctx.enter_context(tc.tile_pool(name="idx", bufs=4))
        segp = ctx.enter_context(tc.tile_pool(name="seg", bufs=6))
        psum = ctx.enter_context(tc.tile_pool(name="psum", bufs=2, space="PSUM"))
        psacc = ctx.enter_context(tc.tile_pool(name="psacc", bufs=2, space="PSUM"))

        nc.gpsimd.load_library(mlp)

        ident = const.tile([P, P], bf16)
        nc.sync.dma_start(out=ident[:], in_=ident_in[:, :])
        riota = const.tile([P, nbmax, P], bf16)
        nc.sync.dma_start(out=riota[:], in_=riota_in[:, :, :])
        dinv_sb = const.tile([P, ntiles], f32)
        nc.sync.dma_start(out=dinv_sb[:], in_=dinv_in[:])
        drel_sb = const.tile([P, total_tok // P], bf16)
        nc.sync.dma_start(out=drel_sb[:], in_=drel_in[:, :])

        w_sb, b_sb = [], []
        for i, (din, dout) in enumerate(dims):
            kcs = math.ceil(din / P)
            wi = []
            for kc in range(kcs):
                rows = min(P, din - kc * P)
                wt = const.tile([rows, dout], bf16, tag=f"w{i}_{kc}")
                nc.sync.dma_start(out=wt[:], in_=Ws[i][kc * P : kc * P + rows, :])
                wi.append(wt)
            w_sb.append(wi)
            bt = const.tile([P, dout], f32, tag=f"b{i}")
            nc.sync.dma_start(out=bt[:], in_=Bs[i][:, :])
            b_sb.append(bt)

        aT = []
        for kc in range(kc0):
            rows = min(P, din0 - kc * P)
            t_ = at_pool.tile([rows, npad], bf16, tag=f"aT_{kc}", name=f"aT0_{kc}")
            nc.sync.dma_start(out=t_[:], in_=xT[kc * P : kc * P + rows, :])
            aT.append(t_)

        n_layers = len(dims)
        for li, (din, dout) in enumerate(dims):
            last = li == n_layers - 1
            kcs = len(aT)

            # ---- local matmul + dinv pre-scale -> bf16 hs table shard ----
            for t in range(ntiles):
                ps = psum.tile([P, dout], f32, tag="mm", name="mm")
                for kc in range(kcs):
                    nc.tensor.matmul(
                        ps[:],
                        aT[kc][:, t * P : (t + 1) * P],
                        w_sb[li][kc][:],
                        start=(kc == 0),
                        stop=(kc == kcs - 1),
                    )
                hs_t = work.tile([P, dout], bf16, tag="hs", name="hs")
                nc.scalar.activation(
                    hs_t[:], ps[:], copyf, bias=0.0, scale=dinv_sb[:, t : t + 1]
                )
                r0 = t * P
                r1 = min((t + 1) * P, npc)
                nc.sync.dma_start(out=hs_loc[li][r0:r1, :], in_=hs_t[0 : r1 - r0, :])

            # ---- AllGather the bf16 table ----
            nc.gpsimd.collective_compute(
                "AllGather",
                mybir.AluOpType.bypass,
                replica_groups=[list(range(n_cores))],
                ins=[hs_loc[li][:, :]],
                outs=[hs_full[li][:, :]],
            )

            # ---- gather + PE segment-sum per dst tile group ----
            if not last:
                kcs_next = math.ceil(dout / P)
                aT_next = []
                for kc in range(kcs_next):
                    rows = min(P, dout - kc * P)
                    aT_next.append(
                        at_pool.tile(
                            [rows, npad],
                            bf16,
                            tag=f"aT_{kc}",
                            name=f"aT{li + 1}_{kc}",
                        )
                    )

            qn = 0
            for grp in layout:
                msgs = {}
                for h, c0, c1 in grp["calls"]:
                    qn = (qn + 1) % 4
                    ntok = c1 - c0
                    idxt = idxp.tile([P, ntok // 16], i16, tag="idxt", name="idxt")
                    nc.sync.dma_start(
                        out=idxt[:], in_=gi_in[:, c0 // 16 : c1 // 16]
                    )
                    msg = msgp.tile([P, ntok // P, dout], bf16, tag="msg", name="msg")
                    lo = h * half
                    rows_h = half if h == 0 else n_nodes - half
                    nc.gpsimd.dma_gather(
                        msg[:],
                        hs_full[li][lo : lo + rows_h, :],
                        idxt[:],
                        ntok,
                        ntok,
                        dout,
                        single_packet=False,
                        queue_num=qn,
                    )
                    msgs[h] = (msg, c0 // P)

                for t, branges in grp["tiles"]:
                    pacc = psacc.tile([P, dout], f32, tag="segacc", name="segacc")
                    segs = []
                    for h, j0, j1 in branges:
                        nblk = j1 - j0
                        segm = segp.tile(
                            [P, nblk, P], bf16, tag="seg", name="seg"
                        )
                        nc.vector.tensor_tensor(
                            out=segm[:],
                            in0=drel_sb[:, j0:j1].to_broadcast([P, nblk, P]),
                            in1=riota[:, 0:nblk, :],
                            op=iseq,
                        )
                        segs.append((h, j0, j1, segm))
                    nb_tot = sum(j1 - j0 for _, j0, j1, _ in segs)
                    bi = 0
                    for h, j0, j1, segm in segs:
                        msg, base = msgs[h]
                        for j in range(j0, j1):
                            nc.tensor.matmul(
                                pacc[:],
                                segm[:, j - j0, :],
                                msg[:, j - base, :],
                                start=(bi == 0),
                                stop=(bi == nb_tot - 1),
                            )
                            bi += 1

                    # ---- epilogue straight from PSUM ----
                    y = work.tile([P, dout], f32, tag="y", name="y")
                    nc.vector.scalar_tensor_tensor(
                        y[:], pacc[:], dinv_sb[:, t : t + 1], b_sb[li][:], mult, add
                    )
                    if last:
                        r0 = t * P
                        r1 = min((t + 1) * P, npc)
                        nc.sync.dma_start(
                            out=out_ext[r0:r1, :], in_=y[0 : r1 - r0, :]
                        )
                    else:
                        a_t = work.tile([P, dout], bf16, tag="a", name="a")
                        nc.scalar.activation(a_t[:], y[:], relu)
                        for kc in range(kcs_next):
                            wcols = min(P, dout - kc * P)
                            pt = psum.tile([wcols, P], bf16, tag="tr", name="tr")
                            nc.tensor.transpose(
                                pt[:], a_t[:, kc * P : kc * P + wcols], ident[:]
                            )
                            nc.scalar.copy(
                                aT_next[kc][:, t * P : (t + 1) * P], pt[:]
                            )
            if not last:
                aT = aT_next

    nc.compile()
    return nc


def _preprocess(x, edge_index, n_nodes, npc, npad, half, n_cores, dims):
    import ml_dtypes

    src = np.asarray(edge_index[0], dtype=np.int64)
    dst = np.asarray(edge_index[1], dtype=np.int64)
    deg = np.bincount(dst, minlength=n_nodes).astype(np.float32) + 1.0
    dinv = (1.0 / np.sqrt(deg)).astype(np.float32)

    ntiles = npad // P
    per_core = []
    for i in range(n_cores):
        lo = i * npc
        sel = (dst >= lo) & (dst < lo + npc)
        s = np.concatenate([src[sel], np.arange(lo, lo + npc, dtype=np.int64)])
        dr = np.concatenate([dst[sel] - lo, np.arange(npc, dtype=np.int64)])
        tl = dr // P
        hh = (s >= half).astype(np.int64)
        order = np.lexsort((s, hh, tl))
        s, dr, hh, tl = s[order], dr[order], hh[order], tl[order]
        key = tl * 2 + hh
        starts = np.searchsorted(key, np.arange(ntiles * 2), "left")
        ends = np.searchsorted(key, np.arange(ntiles * 2), "right")
        per_core.append((s, dr, starts, ends))

    seglen = np.zeros((ntiles, 2), np.int64)
    for s, dr, starts, ends in per_core:
        ln = (ends - starts).reshape(ntiles, 2)
        seglen = np.maximum(seglen, ln)
    seglen = ((seglen + P - 1) // P) * P

    # stream layout: groups of GROUP tiles, within a group h0 segments then h1
    layout = []
    pos = 0
    for g0 in range(0, ntiles, GROUP):
        tl_list = list(range(g0, min(g0 + GROUP, ntiles)))
        calls = []
        tiles = [[t, []] for t in tl_list]
        for h in (0, 1):
            c0 = pos
            for k, t in enumerate(tl_list):
                L = int(seglen[t, h])
                if L:
                    tiles[k][1].append((h, pos // P, (pos + L) // P))
                pos += L
            if pos > c0:
                calls.append((h, c0, pos))
        layout.append({"calls": calls, "tiles": [(t, br) for t, br in tiles]})
    total_tok = pos

    nbmax = max(
        (j1 - j0)
        for grp in layout
        for _, branges in grp["tiles"]
        for _, j0, j1 in branges
    )
    in_maps = []
    for i in range(n_cores):
        s, dr, starts, ends = per_core[i]
        gidx = np.zeros(total_tok, np.int16)
        drel = np.full(total_tok, -1.0, np.float32)
        for grp in layout:
            for t, branges in grp["tiles"]:
                for h, j0, j1 in branges:
                    st, en = starts[t * 2 + h], ends[t * 2 + h]
                    n = en - st
                    p0 = j0 * P
                    gidx[p0 : p0 + n] = (s[st:en] - h * half).astype(np.int16)
                    drel[p0 : p0 + n] = (dr[st:en] - t * P).astype(np.float32)
        lo = i * npc
        x_loc = np.asarray(x[lo : lo + npc], dtype=np.float32)
        xT = np.zeros((x.shape[1], npad), dtype=ml_dtypes.bfloat16)
        xT[:, :npc] = x_loc.T.astype(ml_dtypes.bfloat16)
        dv = np.ones(npad, dtype=np.float32)
        dv[:npc] = dinv[lo : lo + npc]
        in_maps.append(
            {
                "xT": xT,
                "ident": np.eye(P, dtype=ml_dtypes.bfloat16),
                "riota": np.broadcast_to(
                    np.arange(P, dtype=np.float32), (P, nbmax, P)
                ).astype(ml_dtypes.bfloat16),
                "dinv": np.ascontiguousarray(dv.reshape(ntiles, P).T),
                "gidx": np.tile(
                    np.ascontiguousarray(gidx.reshape(total_tok // 16, 16).T),
                    (8, 1),
                ),
                "drel": np.ascontiguousarray(
                    drel.reshape(total_tok // P, P).T
                ).astype(ml_dtypes.bfloat16),
            }
        )
    return in_maps, layout, total_tok, dinv


def _pad_w(w, din_p, dout_p):
    out = np.zeros((din_p, dout_p), np.float32)
    out[: w.shape[0], : w.shape[1]] = w
    return out


def kernel(x, edge_index, W1, b1, W2, b2, W3, b3, W4, b4, **_unused):
    import ml_dtypes
    from concourse.bass_utils import run_bass_kernel_spmd

    x = np.asarray(x, dtype=np.float32)
    n_nodes = x.shape[0]
    npc = n_nodes // N_CORES
    ntiles = math.ceil(npc / P)
    npad = ntiles * P
    half = (n_nodes + 1) // 2

    ws_raw = [np.asarray(w, np.float32) for w in (W1, W2, W3, W4)]
    bs_raw = [np.asarray(b, np.float32) for b in (b1, b2, b3, b4)]
    # pad every dim (except the first input / last output) to a multiple
    # of 128 so bf16 gather elem sizes stay multiples of 256B
    d_in = [ws_raw[0].shape[0]] + [
        max(P, math.ceil(w.shape[1] / P) * P) for w in ws_raw[:-1]
    ]
    d_last = max(P, math.ceil(ws_raw[-1].shape[1] / P) * P)
    d_out = d_in[1:] + [d_last]
    dims = list(zip(d_in, d_out))
    dout_raw = ws_raw[-1].shape[1]
    ws = [
        _pad_w(w, di, do).astype(ml_dtypes.bfloat16)
        for w, (di, do) in zip(ws_raw, dims)
    ]
    bs = [
        np.pad(b, (0, do - b.shape[0])).astype(np.float32)
        for b, (_, do) in zip(bs_raw, dims)
    ]

    in_maps, layout, total_tok, _ = _preprocess(
        x, edge_index, n_nodes, npc, npad, half, N_CORES, dims
    )
    key = (n_nodes, tuple(dims), total_tok)
    if key not in _CACHE:
        _CACHE[key] = _build_gcn(
            n_nodes, npc, npad, half, dims, layout, total_tok, N_CORES
        )
    nc = _CACHE[key]

    for m in in_maps:
        for i in range(4):
            m[f"w{i}"] = ws[i]
            m[f"b{i}"] = np.broadcast_to(bs[i], (P, bs[i].shape[0])).copy()

    import os

    if os.environ.get("GCN_SIM"):
        from concourse.bass_interp import MultiCoreSim

        sim = MultiCoreSim(nc, N_CORES)
        for i in range(N_CORES):
            for k, v in in_maps[i].items():
                sim.cores[i].tensor(k)[:] = v
        sim.simulate(check_with_hw=False)
        return np.concatenate(
            [sim.cores[i].mem_tensor("out") for i in range(N_CORES)], axis=0
        )[:, :dout_raw]

    trace = bool(os.environ.get("GCN_TRACE"))
    res = run_bass_kernel_spmd(
        nc, in_maps, core_ids=list(range(N_CORES)), trace=trace
    )
    global LAST_EXEC_NS
    LAST_EXEC_NS = res.exec_time_ns
    return np.concatenate(
        [res.results[i]["out"] for i in range(N_CORES)], axis=0
    )[:, :dout_raw]


# revision 26
# speedup vs baseline: 1053.9265x; 1.0377x over previous
"""AnomalyDAE 4-layer GCN on 8 TRN2 NeuronCores.

Strategy (node sharding per the sharding hint):
  - Nodes partitioned contiguously across 8 cores (6250/core).
  - Per layer: local matmul h = A @ W (PE, bf16 in / f32 acc), pre-scaled by
    dinv so symmetric normalization becomes out = dinv * segsum(hs[src]),
    hs = dinv * h.  AllGather the bf16 hs table -> [N, dout] per core.
  - Message passing: edges (incl. self-loops) are sorted by destination
    tile (128 dst rows) on host.  Per tile: dma_gather the src rows (bf16)
    from the table, then segment-sum ON THE TENSOR ENGINE: for each
    128-token block, a one-hot selection matrix Seg[tok, row] =
    (dst_rel[tok] == row) is generated by the vector engine (is_equal
    against a row-iota), and matmul(Seg^T @ msg) accumulates the tile's
    [128, dout] result in PSUM.  No scatter-add (its HBM CCE add is not
    atomic for duplicate indices), no accumulator round-trip.
  - Epilogue straight from PSUM: y = dinv*psum + b, relu, PE-transpose to
    build the next layer's lhsT.
  - int16 gather indices => the feature table is split in two halves (per
    core: rows [0,3200) -> table A, rest -> table B), each AllGathered by a
    SEPARATE collective so half-A gathers overlap the half-B collective.
    The d=64 layer is zero-padded to 128 features (bf16 elem size must be
    a multiple of 256 bytes).
"""

import math

import numpy as np

N_CORES = 8
P = 128
GROUP = 2  # dst tiles per gather call pair

_CACHE = {}
LAST_EXEC_NS = None


def _build_gcn(n_nodes, npc, npad, half, dims, layout, total_tok, n_cores):
    import concourse.bacc as bacc
    import concourse.tile as tile
    from concourse import mybir
    from concourse.library_config import mlp
    from contextlib import ExitStack

    f32 = mybir.dt.float32
    bf16 = mybir.dt.bfloat16
    i16 = mybir.dt.int16
    ntiles = npad // P

    nc = bacc.Bacc(
        "TRN2", debug=False, num_devices=n_cores, num_swdge_queues=4
    )

    din0 = dims[0][0]
    kc0 = math.ceil(din0 / P)
    xT = nc.declare_dram_parameter("xT", [din0, npad], bf16, isOutput=False)
    Ws = [
        nc.declare_dram_parameter(f"w{i}", [din, dout], bf16, isOutput=False)
        for i, (din, dout) in enumerate(dims)
    ]
    Bs = [
        nc.declare_dram_parameter(f"b{i}", [P, dout], f32, isOutput=False)
        for i, (din, dout) in enumerate(dims)
    ]
    dinv_in = nc.declare_dram_parameter("dinv", [P, ntiles], f32, isOutput=False)
    ident_in = nc.declare_dram_parameter("ident", [P, P], bf16, isOutput=False)
    nbmax = max(
        (j1 - j0)
        for grp in layout
        for _, branges in grp["tiles"]
        for _, j0, j1 in branges
    )
    riota_in = nc.declare_dram_parameter(
        "riota", [P, nbmax, P], bf16, isOutput=False
    )
    gi_in = nc.declare_dram_parameter(
        "gidx", [P, total_tok // 16], i16, isOutput=False
    )
    drel_in = nc.declare_dram_parameter(
        "drel", [P, total_tok // P], bf16, isOutput=False
    )
    dout_last = dims[-1][1]
    out_ext = nc.declare_dram_parameter("out", [npc, dout_last], f32, isOutput=True)

    split_t = (ntiles + 1) // 2
    rows_a = split_t * P            # per-core rows in half A (tile-aligned)
    rows_b = npc - rows_a           # per-core rows in half B
    hs_loc = [
        (
            nc.dram_tensor(f"hs_locA{i}", [rows_a, d], bf16),
            nc.dram_tensor(f"hs_locB{i}", [rows_b, d], bf16),
        )
        for i, (_, d) in enumerate(dims)
    ]
    hs_full = [
        (
            nc.dram_tensor(
                f"hs_fullA{i}", [n_cores * rows_a, d], bf16, addr_space="Shared"
            ),
            nc.dram_tensor(
                f"hs_fullB{i}", [n_cores * rows_b, d], bf16, addr_space="Shared"
            ),
        )
        for i, (_, d) in enumerate(dims)
    ]

    relu = mybir.ActivationFunctionType.Relu
    copyf = mybir.ActivationFunctionType.Copy
    mult = mybir.AluOpType.mult
    add = mybir.AluOpType.add
    iseq = mybir.AluOpType.is_equal

    with tile.TileContext(nc) as tc, ExitStack() as ctx:
        const = ctx.enter_context(tc.tile_pool(name="const", bufs=1))
        at_pool = ctx.enter_context(tc.tile_pool(name="acts", bufs=2))
        work = ctx.enter_context(tc.tile_pool(name="work", bufs=4))
        msgp = ctx.enter_context(tc.tile_pool(name="msg", bufs=4))
        idxp = ctx.enter_context(tc.tile_pool(name="idx", bufs=4))
        segp = ctx.enter_context(tc.tile_pool(name="seg", bufs=6))
        psum = ctx.enter_context(tc.tile_pool(name="psum", bufs=2, space="PSUM"))
        psacc = ctx.enter_context(tc.tile_pool(name="psacc", bufs=2, space="PSUM"))

        nc.gpsimd.load_library(mlp)

        ident = const.tile([P, P], bf16)
        nc.sync.dma_start(out=ident[:], in_=ident_in[:, :])
        riota = const.tile([P, nbmax, P], bf16)
        nc.sync.dma_start(out=riota[:], in_=riota_in[:, :, :])
        dinv_sb = const.tile([P, ntiles], f32)
        nc.sync.dma_start(out=dinv_sb[:], in_=dinv_in[:])
        drel_sb = const.tile([P, total_tok // P], bf16)
        nc.sync.dma_start(out=drel_sb[:], in_=drel_in[:, :])

        w_sb, b_sb = [], []
        for i, (din, dout) in enumerate(dims):
            kcs = math.ceil(din / P)
            wi = []
            for kc in range(kcs):
                rows = min(P, din - kc * P)
                wt = const.tile([rows, dout], bf16, tag=f"w{i}_{kc}")
                nc.sync.dma_start(out=wt[:], in_=Ws[i][kc * P : kc * P + rows, :])
                wi.append(wt)
            w_sb.append(wi)
            bt = const.tile([P, dout], f32, tag=f"b{i}")
            nc.sync.dma_start(out=bt[:], in_=Bs[i][:, :])
            b_sb.append(bt)

        aT = []
        for kc in range(kc0):
            rows = min(P, din0 - kc * P)
            t_ = at_pool.tile([rows, npad], bf16, tag=f"aT_{kc}", name=f"aT0_{kc}")
            nc.sync.dma_start(out=t_[:], in_=xT[kc * P : kc * P + rows, :])
            aT.append(t_)

        n_layers = len(dims)
        for li, (din, dout) in enumerate(dims):
            last = li == n_layers - 1
            kcs = len(aT)

            # ---- local matmul + dinv pre-scale -> bf16 hs table shard ----
            for t in range(ntiles):
                ps = psum.tile([P, dout], f32, tag="mm", name="mm")
                for kc in range(kcs):
                    nc.tensor.matmul(
                        ps[:],
                        aT[kc][:, t * P : (t + 1) * P],
                        w_sb[li][kc][:],
                        start=(kc == 0),
                        stop=(kc == kcs - 1),
                    )
                hs_t = work.tile([P, dout], bf16, tag="hs", name="hs")
                nc.scalar.activation(
                    hs_t[:], ps[:], copyf, bias=0.0, scale=dinv_sb[:, t : t + 1]
                )
                if t < split_t:
                    r0 = t * P
                    r1 = min((t + 1) * P, rows_a)
                    dst = hs_loc[li][0]
                else:
                    r0 = t * P - rows_a
                    r1 = min((t + 1) * P - rows_a, rows_b)
                    dst = hs_loc[li][1]
                nc.sync.dma_start(out=dst[r0:r1, :], in_=hs_t[0 : r1 - r0, :])

            # ---- AllGather the bf16 table (split so half-A gathers can
            # start while half-B is still in flight) ----
            for hh in (0, 1):
                nc.gpsimd.collective_compute(
                    "AllGather",
                    mybir.AluOpType.bypass,
                    replica_groups=[list(range(n_cores))],
                    ins=[hs_loc[li][hh][:, :]],
                    outs=[hs_full[li][hh][:, :]],
                )

            # ---- gather + PE segment-sum per dst tile group ----
            if not last:
                kcs_next = math.ceil(dout / P)
                aT_next = []
                for kc in range(kcs_next):
                    rows = min(P, dout - kc * P)
                    aT_next.append(
                        at_pool.tile(
                            [rows, npad],
                            bf16,
                            tag=f"aT_{kc}",
                            name=f"aT{li + 1}_{kc}",
                        )
                    )

            qn = 0
            for grp in layout:
                msgs = {}
                for h, c0, c1 in grp["calls"]:
                    qn = (qn + 1) % 4
                    ntok = c1 - c0
                    idxt = idxp.tile([P, ntok // 16], i16, tag="idxt", name="idxt")
                    nc.sync.dma_start(
                        out=idxt[:], in_=gi_in[:, c0 // 16 : c1 // 16]
                    )
                    msg = msgp.tile([P, ntok // P, dout], bf16, tag="msg", name="msg")
                    nc.gpsimd.dma_gather(
                        msg[:],
                        hs_full[li][h][:, :],
                        idxt[:],
                        ntok,
                        ntok,
                        dout,
                        single_packet=False,
                        queue_num=qn,
                    )
                    msgs[h] = (msg, c0 // P)

                for t, branges in grp["tiles"]:
                    pacc = psacc.tile([P, dout], f32, tag="segacc", name="segacc")
                    segs = []
                    for h, j0, j1 in branges:
                        nblk = j1 - j0
                        segm = segp.tile(
                            [P, nblk, P], bf16, tag="seg", name="seg"
                        )
                        nc.vector.tensor_tensor(
                            out=segm[:],
                            in0=drel_sb[:, j0:j1].to_broadcast([P, nblk, P]),
                            in1=riota[:, 0:nblk, :],
                            op=iseq,
                        )
                        segs.append((h, j0, j1, segm))
                    nb_tot = sum(j1 - j0 for _, j0, j1, _ in segs)
                    bi = 0
                    for h, j0, j1, segm in segs:
                        msg, base = msgs[h]
                        for j in range(j0, j1):
                            nc.tensor.matmul(
                                pacc[:],
                                segm[:, j - j0, :],
                                msg[:, j - base, :],
                                start=(bi == 0),
                                stop=(bi == nb_tot - 1),
                            )
                            bi += 1

                    # ---- epilogue straight from PSUM ----
                    y = work.tile([P, dout], f32, tag="y", name="y")
                    nc.vector.scalar_tensor_tensor(
                        y[:], pacc[:], dinv_sb[:, t : t + 1], b_sb[li][:], mult, add
                    )
                    if last:
                        r0 = t * P
                        r1 = min((t + 1) * P, npc)
                        nc.sync.dma_start(
                            out=out_ext[r0:r1, :], in_=y[0 : r1 - r0, :]
                        )
                    else:
                        a_t = work.tile([P, dout], bf16, tag="a", name="a")
                        nc.scalar.activation(a_t[:], y[:], relu)
                        for kc in range(kcs_next):
                            wcols = min(P, dout - kc * P)
                            pt = psum.tile([wcols, P], bf16, tag="tr", name="tr")
                            nc.tensor.transpose(
                                pt[:], a_t[:, kc * P : kc * P + wcols], ident[:]
                            )
                            nc.scalar.copy(
                                aT_next[kc][:, t * P : (t + 1) * P], pt[:]
                            )
            if not last:
                aT = aT_next

    nc.compile()
    return nc


def _preprocess(x, edge_index, n_nodes, npc, npad, half, n_cores, dims):
    import ml_dtypes

    src = np.asarray(edge_index[0], dtype=np.int64)
    dst = np.asarray(edge_index[1], dtype=np.int64)
    deg = np.bincount(dst, minlength=n_nodes).astype(np.float32) + 1.0
    dinv = (1.0 / np.sqrt(deg)).astype(np.float32)

    ntiles = npad // P
    split_t = (ntiles + 1) // 2
    rows_a = split_t * P
    rows_b = npc - rows_a
    per_core = []
    for i in range(n_cores):
        lo = i * npc
        sel = (dst >= lo) & (dst < lo + npc)
        s = np.concatenate([src[sel], np.arange(lo, lo + npc, dtype=np.int64)])
        dr = np.concatenate([dst[sel] - lo, np.arange(npc, dtype=np.int64)])
        tl = dr // P
        owner = s // npc
        rloc = s - owner * npc
        hh = (rloc >= rows_a).astype(np.int64)
        s = np.where(hh == 0, owner * rows_a + rloc, owner * rows_b + rloc - rows_a)
        order = np.lexsort((s, hh, tl))
        s, dr, hh, tl = s[order], dr[order], hh[order], tl[order]
        key = tl * 2 + hh
        starts = np.searchsorted(key, np.arange(ntiles * 2), "left")
        ends = np.searchsorted(key, np.arange(ntiles * 2), "right")
        per_core.append((s, dr, starts, ends))

    seglen = np.zeros((ntiles, 2), np.int64)
    for s, dr, starts, ends in per_core:
        ln = (ends - starts).reshape(ntiles, 2)
        seglen = np.maximum(seglen, ln)
    seglen = ((seglen + P - 1) // P) * P

    # stream layout: groups of GROUP tiles, within a group h0 segments then h1
    layout = []
    pos = 0
    for g0 in range(0, ntiles, GROUP):
        tl_list = list(range(g0, min(g0 + GROUP, ntiles)))
        calls = []
        tiles = [[t, []] for t in tl_list]
        for h in (0, 1):
            c0 = pos
            for k, t in enumerate(tl_list):
                L = int(seglen[t, h])
                if L:
                    tiles[k][1].append((h, pos // P, (pos + L) // P))
                pos += L
            if pos > c0:
                calls.append((h, c0, pos))
        layout.append({"calls": calls, "tiles": [(t, br) for t, br in tiles]})
    total_tok = pos

    nbmax = max(
        (j1 - j0)
        for grp in layout
        for _, branges in grp["tiles"]
        for _, j0, j1 in branges
    )
    in_maps = []
    for i in range(n_cores):
        s, dr, starts, ends = per_core[i]
        gidx = np.zeros(total_tok, np.int16)
        drel = np.full(total_tok, -1.0, np.float32)
        for grp in layout:
            for t, branges in grp["tiles"]:
                for h, j0, j1 in branges:
                    st, en = starts[t * 2 + h], ends[t * 2 + h]
                    n = en - st
                    p0 = j0 * P
                    gidx[p0 : p0 + n] = s[st:en].astype(np.int16)
                    drel[p0 : p0 + n] = (dr[st:en] - t * P).astype(np.float32)
        lo = i * npc
        x_loc = np.asarray(x[lo : lo + npc], dtype=np.float32)
        xT = np.zeros((x.shape[1], npad), dtype=ml_dtypes.bfloat16)
        xT[:, :npc] = x_loc.T.astype(ml_dtypes.bfloat16)
        dv = np.ones(npad, dtype=np.float32)
        dv[:npc] = dinv[lo : lo + npc]
        in_maps.append(
            {
                "xT": xT,
                "ident": np.eye(P, dtype=ml_dtypes.bfloat16),
                "riota": np.broadcast_to(
                    np.arange(P, dtype=np.float32), (P, nbmax, P)
                ).astype(ml_dtypes.bfloat16),
                "dinv": np.ascontiguousarray(dv.reshape(ntiles, P).T),
                "gidx": np.tile(
                    np.ascontiguousarray(gidx.reshape(total_tok // 16, 16).T),
                    (8, 1),
                ),
                "drel": np.ascontiguousarray(
                    drel.reshape(total_tok // P, P).T
                ).astype(ml_dtypes.bfloat16),
            }
        )
    return in_maps, layout, total_tok, dinv


def _pad_w(w, din_p, dout_p):
    out = np.zeros((din_p, dout_p), np.float32)
    out[: w.shape[0], : w.shape[1]] = w
    return out


def kernel(x, edge_index, W1, b1, W2, b2, W3, b3, W4, b4, **_unused):
    import ml_dtypes
    from concourse.bass_utils import run_bass_kernel_spmd

    x = np.asarray(x, dtype=np.float32)
    n_nodes = x.shape[0]
    npc = n_nodes // N_CORES
    ntiles = math.ceil(npc / P)
    npad = ntiles * P
    half = (n_nodes + 1) // 2

    ws_raw = [np.asarray(w, np.float32) for w in (W1, W2, W3, W4)]
    bs_raw = [np.asarray(b, np.float32) for b in (b1, b2, b3, b4)]
    # pad every dim (except the first input / last output) to a multiple
    # of 128 so bf16 gather elem sizes stay multiples of 256B
    d_in = [ws_raw[0].shape[0]] + [
        max(P, math.ceil(w.shape[1] / P) * P) for w in ws_raw[:-1]
    ]
    d_last = max(P, math.ceil(ws_raw[-1].shape[1] / P) * P)
    d_out = d_in[1:] + [d_last]
    dims = list(zip(d_in, d_out))
    dout_raw = ws_raw[-1].shape[1]
    ws = [
        _pad_w(w, di, do).astype(ml_dtypes.bfloat16)
        for w, (di, do) in zip(ws_raw, dims)
    ]
    bs = [
        np.pad(b, (0, do - b.shape[0])).astype(np.float32)
        for b, (_, do) in zip(bs_raw, dims)
    ]

    in_maps, layout, total_tok, _ = _preprocess(
        x, edge_index, n_nodes, npc, npad, half, N_CORES, dims
    )
    key = (n_nodes, tuple(dims), total_tok)
    if key not in _CACHE:
        _CACHE[key] = _build_gcn(
            n_nodes, npc, npad, half, dims, layout, total_tok, N_CORES
        )
    nc = _CACHE[key]

    for m in in_maps:
        for i in range(4):
            m[f"w{i}"] = ws[i]
            m[f"b{i}"] = np.broadcast_to(bs[i], (P, bs[i].shape[0])).copy()

    import os

    if os.environ.get("GCN_SIM"):
        from concourse.bass_interp import MultiCoreSim

        sim = MultiCoreSim(nc, N_CORES)
        for i in range(N_CORES):
            for k, v in in_maps[i].items():
                sim.cores[i].tensor(k)[:] = v
        sim.simulate(check_with_hw=False)
        return np.concatenate(
            [sim.cores[i].mem_tensor("out") for i in range(N_CORES)], axis=0
        )[:, :dout_raw]

    trace = bool(os.environ.get("GCN_TRACE"))
    res = run_bass_kernel_spmd(
        nc, in_maps, core_ids=list(range(N_CORES)), trace=trace
    )
    global LAST_EXEC_NS
    LAST_EXEC_NS = res.exec_time_ns
    return np.concatenate(
        [res.results[i]["out"] for i in range(N_CORES)], axis=0
    )[:, :dout_raw]
